# revision 3
# baseline (speedup 1.0000x reference)
"""CapsNet (EM routing) on 8 Trainium2 NeuronCores.

Strategy: pure data parallel over the batch dim (8 samples -> 8 cores),
all conv / transformation / beta params replicated, per the sharding hint.
Each core runs the full per-sample network (conv1 -> primary caps ->
3x EM-routing caps layers); outputs are gathered on host.

Optimization notes:
- In the reference, `rp * votes / rp` cancels, so means/var/dev2/expo/
  agreement are iteration-independent *except* for NaN propagation
  (rp == 0 or non-finite yields NaN in the reference). We hoist the big
  (I*O*16-sized) tensors out of the routing loop and inject NaNs exactly
  where the reference would via c = 0.0 * sum_i(r/r - 1):
  c is +0.0 when all r are finite & nonzero (x + 0.0 keeps fp bits), and
  NaN otherwise -- reproducing the reference NaN mask bit-for-bit while
  cutting the per-iteration work to small (I*O)-sized tensors.
- Params are device-put once and cached across kernel() calls.
"""

import numpy as np
import jax
import jax.numpy as jnp

A = 32
B = 32
POSE = 4
P2 = POSE * POSE
ROUTING_ITER = 3
EPS = 1e-6
N_CORES = 8


def _caps_layer(pose_in, act_in, T, beta_v, beta_a):
    votes = jax.nn.relu(jnp.einsum('nhwipq,ioqr->nhwiopr', pose_in, T))
    N, H, W, I, O = votes.shape[:5]
    votes = votes.reshape(N, H, W, I, O, P2)

    # ---- iteration-independent heavy tensors (see module docstring) ----
    means0 = jnp.sum(votes, axis=-3)                       # (N,H,W,O,16)
    dev20 = jnp.square(votes - means0[..., None, :, :])    # (N,H,W,I,O,16)
    var0 = jnp.sum(dev20, axis=-3)                         # (N,H,W,O,16)
    expo0 = -jnp.sum(dev20 / (2.0 * var0[..., None, :, :]), axis=-1)  # (N,H,W,I,O)
    coeff_inv0 = jnp.sqrt(jnp.prod(2.0 * jnp.pi * var0, axis=-1))[..., None, :]
    agreement0 = jnp.exp(expo0) / coeff_inv0               # (N,H,W,I,O)

    r = jnp.full((1, 1, 1, I, O), 1.0 / O, dtype=votes.dtype)
    act_out = None
    for it in range(ROUTING_ITER):
        inv_temp = 1.0 + it
        r = r * act_in[..., None]                          # (N,H,W,I,O)
        # NaN injection: c == +0.0 where reference's rp*votes/rp == votes,
        # NaN where the reference would poison means/var (r zero/inf/nan).
        c = 0.0 * jnp.sum(r / r - 1.0, axis=-2)            # (N,H,W,O)
        var = var0 + c[..., None]
        r_sum = jnp.sum(r, axis=-2)[..., None]             # (N,H,W,O,1)
        cost = (beta_v - 0.5 * jnp.log(var)) * r_sum
        act_out = jax.nn.sigmoid(inv_temp * (beta_a - jnp.sum(cost, axis=-1)))
        if it + 1 < ROUTING_ITER:
            agreement = agreement0 + c[..., None, :]
            num = EPS + act_out[..., None, :] * agreement
            r = num / jnp.sum(num, axis=-2, keepdims=True)

    # pose_out must carry the *final* iteration's NaN mask (c from it=2)
    means = means0 + (0.0 * jnp.sum(r / r - 1.0, axis=-2))[..., None]
    pose_out = means.reshape(*means.shape[:-1], POSE, POSE)
    return pose_out, act_out


def _conv(x, w, b, stride, pad):
    y = jax.lax.conv_general_dilated(
        x, w, window_strides=(stride, stride), padding=((pad, pad), (pad, pad)),
        dimension_numbers=('NCHW', 'OIHW', 'NCHW'))
    return y + b[None, :, None, None]


def _forward(x, conv1_w, conv1_b, pconv_w, pconv_b,
             t1, bv1, ba1, t2, bv2, ba2, t3, bv3, ba3):
    y = jax.nn.relu(_conv(x, conv1_w, conv1_b, 2, 2))
    z = _conv(y, pconv_w, pconv_b, 1, 0)
    n, _, h, w = z.shape
    act = z[:, :B].reshape(n, h, w, B)
    pose = z[:, B:].reshape(n, h, w, B, POSE, POSE)
    pose, act = _caps_layer(pose, act, t1, bv1, ba1)
    pose, act = _caps_layer(pose, act, t2, bv2, ba2)
    pose, act = _caps_layer(pose, act, t3, bv3, ba3)
    return act


_CACHE = {}


def kernel(x, conv1_w, conv1_b, pconv_w, pconv_b,
           t1, bv1, ba1, t2, bv2, ba2, t3, bv3, ba3):
    n = x.shape[0]
    assert n == N_CORES, f"expected batch {N_CORES}, got {n}"
    devices = jax.devices()[:N_CORES]
    params = tuple(np.asarray(p, np.float32) for p in
                   (conv1_w, conv1_b, pconv_w, pconv_b,
                    t1, bv1, ba1, t2, bv2, ba2, t3, bv3, ba3))
    key = tuple(p.tobytes() for p in params)
    if _CACHE.get('key') != key:
        _CACHE['key'] = key
        _CACHE['params'] = [jax.device_put_replicated(p, devices) for p in params]
        _CACHE['fn'] = jax.pmap(_forward, devices=devices)
    xs = np.asarray(x, np.float32).reshape(n, 1, *x.shape[1:])
    out = _CACHE['fn'](xs, *_CACHE['params'])   # (8, 1, 14, 14, 10)
    out = np.asarray(out).reshape(n, 14, 14, out.shape[-1])
    return out.astype(np.float32)
